# revision 1
# baseline (speedup 1.0000x reference)
"""Trainium2 Bass kernel for nn_CrossNetwork (3x [BatchNorm1d -> cross update]).

Math per layer (reference):
    mu   = mean(x, axis=0)                  # over batch B
    var  = mean((x-mu)^2, axis=0)           # biased
    xn   = (x - mu) * gamma/sqrt(var+eps) + beta
    s    = xn @ w                           # per-row dot over features L
    x'   = x0 * s[:, None] + b + xn

Sharding: L-shard (feature/model parallel), NOT the batch-parallel hint.
Each of the 8 cores owns 128 of the 1024 features, laid out transposed in
SBUF as [128 feature-partitions, 16384 batch-free].  BatchNorm stats are
then fully core-local (free-dim reductions); the only cross-core exchange
is the AllReduce of the per-core partial dot s (64 KB/layer, split in two
halves to overlap with compute).

b_1, b_2 never need to be materialized: a per-feature additive constant is
removed by the next layer's BatchNorm (mean shift) and does not affect
xn/s of later layers.  Only b_3 appears in the output; it is folded into
layer 3's normalize bias, with the dot corrected by the scalar sum(b3*w3).

Engines per layer:
  DVE    : t = x0*s_bcast (tensor_tensor_reduce, also yields sum(t) for the
           next layer's mean), part of the add, tiny stat/param math
  GPSIMD : bulk of the add x' = t + xn (runs parallel to 1x DVE ops)
  ACT    : normalize (Identity w/ per-partition scale+bias), Square pass
           with accum_out for the next layer's E[x^2]
  PE     : dot partials (w^T @ xn) and s broadcast (ones (x) s), in f32r
  DMA    : HWDGE loads/stores; TOPSP AllReduce for s
"""

import os
import sys

import numpy as np

for _p in ("/opt/trn_rl_repo", "/root/.axon_site/_ro/trn_rl_repo"):
    if os.path.isdir(_p) and _p not in sys.path:
        sys.path.insert(0, _p)
        break

P = 128          # feature partitions per core
NCORES = 8
NL = 3
EPS = 1e-8
MAGIC = 0x5F3759DF
WSCALE = 4096.0


def build_nc(F=16384, n_cores=NCORES, debug=False):
    """Builds + returns the Bacc module (uncompiled). F = batch per core."""
    from contextlib import ExitStack

    import concourse.bacc as bacc
    import concourse.bass_isa as bass_isa
    import concourse.mybir as mybir
    import concourse.tile as tile
    from concourse.alu_op_type import AluOpType as alu

    dt = mybir.dt
    f32 = dt.float32
    f32r = dt.float32r
    i32 = dt.int32
    AF = mybir.ActivationFunctionType
    AX = mybir.AxisListType

    CH = 1024                 # working chunk (free dim)
    NCH = F // CH             # chunks per tensor
    DH = (F // 2) // 512      # 512-wide dot chunks per half
    CPG = max(1, DH // 4)     # dot chunks per partition-group
    NPG = DH // CPG           # partition groups used (<= 4)
    HALF = F // 2

    nc = bacc.Bacc("TRN2", target_bir_lowering=False, debug=debug,
                   num_devices=n_cores)

    xT = nc.dram_tensor("xT", [P, F], f32, kind="ExternalInput").ap()
    par = nc.dram_tensor("par", [P, 12], f32, kind="ExternalInput").ap()
    outT = nc.dram_tensor("outT", [P, F], f32, kind="ExternalOutput").ap()
    bf16 = dt.float16  # fp16: s ~1e-2 fits, 8x finer mantissa than bf16
    cc_in = [nc.dram_tensor(f"cc_in{h}", [HALF], bf16).ap() for h in range(2)]
    cc_out = [nc.dram_tensor(f"cc_out{h}", [HALF], bf16,
                             addr_space="Shared").ap() for h in range(2)]
    rg = [list(range(n_cores))]

    with tile.TileContext(nc) as tc, ExitStack() as ctx:
        big = ctx.enter_context(tc.tile_pool(name="big", bufs=1))
        sm = ctx.enter_context(tc.tile_pool(name="small", bufs=1))
        tpool = ctx.enter_context(tc.tile_pool(name="tmul", bufs=2))
        xhpool = ctx.enter_context(tc.tile_pool(name="xh", bufs=4))
        sqpool = ctx.enter_context(tc.tile_pool(name="sq", bufs=2))
        pdot = ctx.enter_context(tc.tile_pool(name="pdot", bufs=2, space="PSUM"))
        pbc = ctx.enter_context(tc.tile_pool(name="pbc", bufs=2, space="PSUM"))

        # x0 (pristine input) and the working tensor, chunked so Tile can
        # pipeline at chunk granularity (deps are per-tile).
        A = [big.tile([P, CH], f32, tag=f"A{i}", name=f"A{i}") for i in range(NCH)]
        Bt = [big.tile([P, CH], f32, tag=f"B{i}", name=f"B{i}") for i in range(NCH)]
        s_stage = big.tile([1, HALF], bf16, tag="sst", name="sst")
        s_flat = big.tile([1, HALF], bf16, tag="sflat", name="s_flat")

        ones_bf = sm.tile([1, P], bf16)
        w_h = sm.tile([P, NL], bf16)
        par_sb = sm.tile([P, 12], f32)
        bnst = sm.tile([P, F // 512, 6], f32)
        meanvar = sm.tile([P, 2], f32)
        sum_acc = sm.tile([P, NCH], f32)
        ss_acc = sm.tile([P, NCH], f32)
        geff = sm.tile([P, NL], f32)
        cbias = sm.tile([P, NL], f32)
        veps = sm.tile([P, 1], f32)
        rsq = sm.tile([P, 1], f32)
        nt1 = sm.tile([P, 1], f32)
        tsum = sm.tile([P, 1], f32)
        ssum = sm.tile([P, 1], f32)
        mean2 = sm.tile([P, 1], f32)
        var2 = sm.tile([P, 1], f32)
        bw = sm.tile([P, 1], f32)
        corr = sm.tile([P, 1], f32)
        negcorr = sm.tile([P, 1], f32)

        nc.vector.memset(ones_bf[:], 1.0)
        nc.sync.dma_start(par_sb[:], par[:])

        # ---- load + layer-1 stats (bn_stats trails each chunk DMA) ----
        for c in range(NCH):
            sl = slice(c * CH, (c + 1) * CH)
            nc.sync.dma_start(A[c][:], xT[:, sl])
            for j in range(CH // 512):
                d = c * (CH // 512) + j
                nc.vector.bn_stats(bnst[:, d, :],
                                   A[c][:, j * 512:(j + 1) * 512])
        nc.vector.bn_aggr(meanvar[:],
                          bnst[:].rearrange("p a b -> p (a b)"))

        def layer_params(k, mean_ap, var_ap):
            # rsqrt(var+eps) via quake seed + 3 Newton iterations (all DVE,
            # avoids the inaccurate ACT Sqrt and its table load)
            nc.vector.tensor_scalar(veps[:], var_ap, EPS, None, alu.add)
            vi = veps[:].bitcast(i32)
            ri = rsq[:].bitcast(i32)
            nc.vector.tensor_scalar(ri, vi, 1, None, alu.logical_shift_right)
            nc.vector.tensor_scalar(ri, ri, -1, MAGIC, alu.mult, alu.add)
            r = rsq[:]
            for _ in range(3):
                nc.vector.tensor_tensor(nt1[:], r, r, alu.mult)
                nc.vector.tensor_tensor(nt1[:], nt1[:], veps[:], alu.mult)
                nc.vector.tensor_scalar(nt1[:], nt1[:], -0.5, 1.5,
                                        alu.mult, alu.add)
                nc.vector.tensor_tensor(r, r, nt1[:], alu.mult)
            nc.vector.tensor_scalar(w_h[:, k:k + 1], par_sb[:, 6 + k:7 + k],
                                    WSCALE, None, alu.mult)
            nc.vector.tensor_tensor(geff[:, k:k + 1], par_sb[:, k:k + 1], r,
                                    alu.mult)
            nc.vector.tensor_tensor(nt1[:], mean_ap, geff[:, k:k + 1],
                                    alu.mult)
            nc.vector.tensor_tensor(cbias[:, k:k + 1],
                                    par_sb[:, 3 + k:4 + k], nt1[:],
                                    alu.subtract)
            if k == NL - 1:
                # fold b3 into the normalize bias; correct the dot by
                # sum(b3*w3) (this core's share, removed pre-AllReduce)
                nc.vector.tensor_tensor(cbias[:, k:k + 1], cbias[:, k:k + 1],
                                        par_sb[:, 9 + k:10 + k], alu.add)
                nc.vector.tensor_tensor(bw[:], par_sb[:, 9 + k:10 + k],
                                        par_sb[:, 6 + k:7 + k], alu.mult)
                nc.gpsimd.partition_all_reduce(corr[:], bw[:], P,
                                               bass_isa.ReduceOp.add)
                nc.vector.tensor_scalar(negcorr[:], corr[:], -1.0, None,
                                        alu.mult)

        def half_b(k, h):
            """normalize half -> dot partials -> stage -> DMA out -> AllReduce."""
            g_ap = geff[:, k:k + 1]
            c_ap = cbias[:, k:k + 1]
            base = h * (NCH // 2)
            xhs = {}
            for cc in range(NCH // 2):
                c = base + cc
                src = A[c] if k == 0 else Bt[c]
                # fp16 xn for the dots (DVE) must read src before the
                # in-place f32 normalize on ACT overwrites it
                xh = xhpool.tile([P, CH], bf16, tag="xh", name="xh")
                nc.vector.tensor_scalar(xh[:], src[:], g_ap, c_ap,
                                        alu.mult, alu.add)
                nc.scalar.activation(Bt[c][:], src[:], AF.Identity,
                                     bias=c_ap, scale=g_ap)
                xhs[c] = xh
            # dots in pairs of 512 -> [1, 1024] psum at partition 0,
            # drained (with optional b3*w3 correction) to fp16 s_stage,
            # alternating DVE/ACT to split the single-lane drain cost
            for e in range(DH // 2):
                pd = pdot.tile([1, 1024], f32, tag="pd", name="pd")
                for i in range(2):
                    dl = 2 * e + i
                    c = (h * DH + dl) // (CH // 512)
                    j = (h * DH + dl) % (CH // 512)
                    rhs = xhs[c][:, j * 512:(j + 1) * 512]
                    nc.tensor.matmul(pd[0:1, i * 512:(i + 1) * 512],
                                     w_h[:, k:k + 1], rhs,
                                     start=True, stop=True)
                dst = s_stage[0:1, e * 1024:(e + 1) * 1024]
                if k == NL - 1:
                    if e % 2 == 0:
                        nc.vector.tensor_scalar(dst, pd[0:1, :],
                                                1.0 / WSCALE, corr[0:1, :],
                                                alu.mult, alu.subtract)
                    else:
                        nc.scalar.activation(dst, pd[0:1, :], AF.Identity,
                                             bias=negcorr[0:1, :],
                                             scale=1.0 / WSCALE)
                else:
                    if e % 2 == 0:
                        nc.vector.tensor_scalar(dst, pd[0:1, :],
                                                1.0 / WSCALE, None, alu.mult)
                    else:
                        nc.scalar.mul(dst, pd[0:1, :], 1.0 / WSCALE)
            nc.sync.dma_start(cc_in[h].rearrange("(o e) -> o e", o=1),
                              s_stage[:])
            nc.gpsimd.collective_compute(
                "AllReduce", mybir.AluOpType.add, replica_groups=rg,
                ins=[cc_in[h]], outs=[cc_out[h]])

        def readback(h):
            nc.sync.dma_start(s_flat[:],
                              cc_out[h].rearrange("(o e) -> o e", o=1))

        def half_c(k, h):
            """x' = x0 * s_bcast + xn  (+ next-layer stats / final store)."""
            last = k == NL - 1
            base = h * (NCH // 2)
            for cc in range(NCH // 2):
                c = base + cc
                pb = pbc.tile([P, CH], f32, tag="pb", name="pb")
                off0 = (c * CH) % HALF
                for i in range(CH // 512):
                    nc.tensor.matmul(
                        pb[:, i * 512:(i + 1) * 512], ones_bf[0:1, :],
                        s_flat[0:1, off0 + i * 512:off0 + (i + 1) * 512],
                        start=True, stop=True)
                tt = tpool.tile([P, CH], bf16, tag="tt", name="tt")
                nc.vector.tensor_tensor(tt[:], A[c][:], pb[:], alu.mult)
                nc.vector.tensor_scalar(tt[:], tt[:], 1.0, 0.0, alu.mult,
                                        alu.add, accum_out=sum_acc[:, c:c + 1])
                eng = nc.vector if c % 4 == 3 else nc.gpsimd
                eng.tensor_tensor(Bt[c][:], Bt[c][:], tt[:], alu.add)
                if last:
                    nc.sync.dma_start(outT[:, c * CH:(c + 1) * CH], Bt[c][:])
                else:
                    sq = sqpool.tile([P, CH], f32, tag="sqs", name="sqs")
                    nc.scalar.activation(sq[:], Bt[c][:], AF.Square,
                                         accum_out=ss_acc[:, c:c + 1])

        def finalize_stats(k):
            """stats of x_{k+1}: mean from sum(t)+B*beta_k, var from E[x^2]."""
            nc.vector.tensor_reduce(tsum[:], sum_acc[:], AX.X, alu.add)
            nc.vector.tensor_reduce(ssum[:], ss_acc[:], AX.X, alu.add)
            nc.vector.tensor_scalar(mean2[:], tsum[:], 1.0 / F,
                                    par_sb[:, 3 + k:4 + k], alu.mult, alu.add)
            nc.vector.tensor_scalar(ssum[:], ssum[:], 1.0 / F, None, alu.mult)
            nc.vector.tensor_tensor(nt1[:], mean2[:], mean2[:], alu.mult)
            nc.vector.tensor_tensor(var2[:], ssum[:], nt1[:], alu.subtract)

        mean_ap, var_ap = meanvar[:, 0:1], meanvar[:, 1:2]
        for k in range(NL):
            layer_params(k, mean_ap, var_ap)
            half_b(k, 0)
            half_b(k, 1)       # AR(h1) flies while C(h0) runs
            readback(0)
            half_c(k, 0)
            readback(1)        # WAR: waits for C(h0)'s s_flat readers
            half_c(k, 1)
            if k < NL - 1:
                finalize_stats(k)
                mean_ap, var_ap = mean2[:], var2[:]

    return nc


_CACHE = {}


def _get_compiled():
    if "nc" not in _CACHE:
        nc = build_nc()
        nc.compile()
        _CACHE["nc"] = nc
    return _CACHE["nc"]


def kernel(x, gamma, beta, w, b):
    from concourse.bass_utils import run_bass_kernel_spmd

    x = np.asarray(x, dtype=np.float32)
    gamma = np.asarray(gamma, dtype=np.float32)
    beta = np.asarray(beta, dtype=np.float32)
    w = np.asarray(w, dtype=np.float32)
    b = np.asarray(b, dtype=np.float32)
    B_, L_ = x.shape

    nc = _get_compiled()
    in_maps = []
    for c in range(NCORES):
        cols = slice(c * P, (c + 1) * P)
        in_maps.append({
            "xT": np.ascontiguousarray(x[:, cols].T),
            "par": np.ascontiguousarray(np.concatenate(
                [gamma[:, cols].T, beta[:, cols].T,
                 w[:, cols].T, b[:, cols].T], axis=1)),
        })
    res = run_bass_kernel_spmd(nc, in_maps, list(range(NCORES))).results
    out = np.empty((B_, L_), np.float32)
    for c in range(NCORES):
        out[:, c * P:(c + 1) * P] = res[c]["outT"].T
    return out



# revision 4
# speedup vs baseline: 1.0925x; 1.0925x over previous
"""Trainium2 Bass kernel for nn_CrossNetwork (3x [BatchNorm1d -> cross update]).

Math per layer (reference):
    mu   = mean(x, axis=0)                  # over batch B
    var  = mean((x-mu)^2, axis=0)           # biased
    xn   = (x - mu) * gamma/sqrt(var+eps) + beta
    s    = xn @ w                           # per-row dot over features L
    x'   = x0 * s[:, None] + b + xn

Sharding: L-shard (feature/model parallel). Each of the 8 cores owns 128
of the 1024 features, laid out transposed in SBUF as [128 feature
partitions, 16384 batch free]. BatchNorm stats are core-local free-dim
reductions; the only cross-core exchange is the AllReduce of the
per-core partial dot s (fp16, one per half per layer).

Restructure vs the straightforward version: xn is never materialized.
With g = gamma*rsqrt(var+eps) and c = beta - mu*g, per layer
    x_{k+1} = g_k x_k + (c_k + b_k) + x0 * s_k .
Split the per-partition constant into a deferred offset D (D_0 = 0,
D_{k+1} = g_k D_k + c_k + b_k); the stored tensor follows
    X_{k+1} = g_k X_k + t_k,   t_k = x0 * s_k,   true x_k = X_k + D_k.
The dot then reads the raw X_k with folded weights:
    s_k = sum_p (w g)[p] X_k[p,:] + share_k,
    share_k = sum_p w[p] (g[p] D_k[p] + c_k[p])   (batch-independent),
where each core adds its own share at PSUM drain time, pre-AllReduce
(the AllReduce sums the shares to the global constant). Stats come from
accumulators: Sum(X') rides the post-add fp16 identity pass (DVE 4x
mode), Sum(X'^2) rides the ACT Square pass; mean_true = Sum(X')/B + D,
var = Sum(X'^2)/B - (Sum(X')/B)^2 (shift-invariant). The final layer's
output pass on ACT applies +D_3 for free and writes f32 directly.

Everything bulk is fp16 (tolerance 2e-2 leaves ~30x slack): DVE
tensor_scalar runs 4x, tensor_tensor 2x. The all-reduced s is
partition-broadcast into SBUF by DMA (stride-0 source AP), so the
t-pass never touches PSUM and keeps its fast mode.
"""

import os
import sys

import numpy as np

for _p in ("/opt/trn_rl_repo", "/root/.axon_site/_ro/trn_rl_repo"):
    if os.path.isdir(_p) and _p not in sys.path:
        sys.path.insert(0, _p)
        break

P = 128          # feature partitions per core
NCORES = 8
NL = 3
EPS = 1e-8
MAGIC = 0x5F3759DF
WSCALE = 4096.0   # keeps w (~1e-4) out of fp16 subnormal range in the lhs


def build_nc(F=16384, n_cores=NCORES, debug=False):
    """Builds + returns the Bacc module (uncompiled). F = batch per core."""
    from contextlib import ExitStack

    import concourse.bacc as bacc
    import concourse.bass_isa as bass_isa
    import concourse.mybir as mybir
    import concourse.tile as tile
    from concourse.alu_op_type import AluOpType as alu

    dt = mybir.dt
    f32 = dt.float32
    fp16 = dt.float16
    i32 = dt.int32
    AF = mybir.ActivationFunctionType
    AX = mybir.AxisListType

    CH = 1024                 # working chunk (free dim)
    NCH = F // CH             # chunks per tensor
    HALF = F // 2
    HCH = NCH // 2            # chunks per half
    invW = 1.0 / WSCALE

    nc = bacc.Bacc("TRN2", target_bir_lowering=False, debug=debug,
                   num_devices=n_cores)

    xT = nc.dram_tensor("xT", [P, F], f32, kind="ExternalInput").ap()
    par = nc.dram_tensor("par", [P, 12], f32, kind="ExternalInput").ap()
    outT = nc.dram_tensor("outT", [P, F], f32, kind="ExternalOutput").ap()
    cc_in = [nc.dram_tensor(f"cc_in{h}", [HALF], fp16).ap() for h in range(2)]
    cc_out = [nc.dram_tensor(f"cc_out{h}", [HALF], fp16,
                             addr_space="Shared").ap() for h in range(2)]
    rg = [list(range(n_cores))]

    with tile.TileContext(nc) as tc, ExitStack() as ctx:
        big = ctx.enter_context(tc.tile_pool(name="big", bufs=1))
        sm = ctx.enter_context(tc.tile_pool(name="small", bufs=1))
        io = ctx.enter_context(tc.tile_pool(name="io", bufs=4))
        tp = ctx.enter_context(tc.tile_pool(name="tmul", bufs=4))
        pdot = ctx.enter_context(tc.tile_pool(name="pdot", bufs=2, space="PSUM"))

        # persistent big tiles, chunked so Tile pipelines at chunk grain
        X0 = [big.tile([P, CH], fp16, tag=f"X0_{i}", name=f"X0_{i}")
              for i in range(NCH)]
        X = [big.tile([P, CH], fp16, tag=f"X_{i}", name=f"X_{i}")
             for i in range(NCH)]
        s_sb = [big.tile([P, HALF], fp16, tag=f"ssb{h}", name=f"ssb{h}")
                for h in range(2)]
        stg = [big.tile([1, HALF], fp16, tag=f"stg{h}", name=f"stg{h}")
               for h in range(2)]
        edump = big.tile([P, CH], fp16, tag="edump", name="edump")
        fdump = big.tile([P, CH], fp16, tag="fdump", name="fdump")

        par_sb = sm.tile([P, 12], f32)
        w_h = sm.tile([P, NL], fp16)
        ts_acc = sm.tile([P, NCH], f32)
        sq_acc = sm.tile([P, NCH], f32)
        geff = sm.tile([P, 1], f32)
        cbias = sm.tile([P, 1], f32)
        Dof = sm.tile([P, 1], f32)
        veps = sm.tile([P, 1], f32)
        rsq = sm.tile([P, 1], f32)
        nt1 = sm.tile([P, 1], f32)
        nt2 = sm.tile([P, 1], f32)
        shrv = sm.tile([P, 1], f32)
        tsum = sm.tile([P, 1], f32)
        ssum = sm.tile([P, 1], f32)
        mean_t = sm.tile([P, 1], f32)
        var_t = sm.tile([P, 1], f32)

        nc.sync.dma_start(par_sb[:], par[:])
        nc.vector.memset(Dof[:], 0.0)

        # ---- load: DMA f32 chunk -> fp16 convert (DVE, accum Sum x) ----
        # ----       -> Square (ACT, accum Sum x^2)                   ----
        for c in range(NCH):
            sl = slice(c * CH, (c + 1) * CH)
            ld = io.tile([P, CH], f32, tag="io", name="io")
            nc.sync.dma_start(ld[:], xT[:, sl])
            nc.vector.tensor_scalar(X0[c][:], ld[:], 1.0, 0.0,
                                    alu.mult, alu.add,
                                    accum_out=ts_acc[:, c:c + 1])
            nc.scalar.activation(fdump[:], X0[c][:], AF.Square,
                                 accum_out=sq_acc[:, c:c + 1])

        def finalize_stats():
            """mean/var of stored X from the chunk accumulators."""
            nc.vector.tensor_reduce(tsum[:], ts_acc[:], AX.X, alu.add)
            nc.vector.tensor_reduce(ssum[:], sq_acc[:], AX.X, alu.add)
            # m_st = Sum(X)/F ; mean_true = m_st + D ; var = Sum(X^2)/F - m_st^2
            nc.vector.tensor_scalar(tsum[:], tsum[:], 1.0 / F, None, alu.mult)
            nc.vector.tensor_tensor(mean_t[:], tsum[:], Dof[:], alu.add)
            nc.vector.tensor_tensor(nt1[:], tsum[:], tsum[:], alu.mult)
            nc.vector.tensor_scalar(ssum[:], ssum[:], 1.0 / F, None, alu.mult)
            nc.vector.tensor_tensor(var_t[:], ssum[:], nt1[:], alu.subtract)

        finalize_stats()

        def layer_params(k):
            # rsqrt(var+eps): quake seed + 3 Newton iterations (all DVE)
            nc.vector.tensor_scalar(veps[:], var_t[:], EPS, None, alu.add)
            vi = veps[:].bitcast(i32)
            ri = rsq[:].bitcast(i32)
            nc.vector.tensor_scalar(ri, vi, 1, None, alu.logical_shift_right)
            nc.vector.tensor_scalar(ri, ri, -1, MAGIC, alu.mult, alu.add)
            r = rsq[:]
            for _ in range(3):
                nc.vector.tensor_tensor(nt1[:], r, r, alu.mult)
                nc.vector.tensor_tensor(nt1[:], nt1[:], veps[:], alu.mult)
                nc.vector.tensor_scalar(nt1[:], nt1[:], -0.5, 1.5,
                                        alu.mult, alu.add)
                nc.vector.tensor_tensor(r, r, nt1[:], alu.mult)
            # g = gamma * rsqrt ; c = beta - mean_true * g
            nc.vector.tensor_tensor(geff[:], par_sb[:, k:k + 1], r, alu.mult)
            nc.vector.tensor_tensor(nt1[:], mean_t[:], geff[:], alu.mult)
            nc.vector.tensor_tensor(cbias[:], par_sb[:, 3 + k:4 + k], nt1[:],
                                    alu.subtract)
            # share = sum_p w*(g*D + c)  (this core's slice; AR sums them)
            nc.vector.tensor_tensor(nt2[:], geff[:], Dof[:], alu.mult)
            nc.vector.tensor_tensor(nt2[:], nt2[:], cbias[:], alu.add)
            nc.vector.tensor_tensor(nt2[:], nt2[:], par_sb[:, 6 + k:7 + k],
                                    alu.mult)
            nc.gpsimd.partition_all_reduce(shrv[:], nt2[:], P,
                                           bass_isa.ReduceOp.add)
            # folded dot weights: (w*g)*WSCALE, fp16 lhs
            nc.vector.tensor_tensor(nt1[:], par_sb[:, 6 + k:7 + k], geff[:],
                                    alu.mult)
            nc.vector.tensor_scalar(w_h[:, k:k + 1], nt1[:], WSCALE, None,
                                    alu.mult)
            # D' = g*D + (c + b)   (after share, which uses the old D)
            nc.vector.tensor_tensor(nt1[:], cbias[:],
                                    par_sb[:, 9 + k:10 + k], alu.add)
            nc.vector.tensor_scalar(Dof[:], Dof[:], geff[:], nt1[:],
                                    alu.mult, alu.add)

        def half_dots(k, h, src):
            """dots -> drain (+share, /WSCALE, fp16) -> cc DMA -> AllReduce."""
            for cc in range(HCH):
                c = h * HCH + cc
                pd = pdot.tile([1, CH], f32, tag="pd", name="pd")
                for j in range(CH // 512):
                    rhs = src[c][:, j * 512:(j + 1) * 512]
                    nc.tensor.matmul(pd[0:1, j * 512:(j + 1) * 512],
                                     w_h[:, k:k + 1], rhs,
                                     start=True, stop=True)
                dst = stg[h][0:1, cc * CH:(cc + 1) * CH]
                if cc % 4 == 0:
                    nc.vector.tensor_scalar(dst, pd[0:1, :], invW,
                                            shrv[0:1, :], alu.mult, alu.add)
                else:
                    nc.scalar.activation(dst, pd[0:1, :], AF.Identity,
                                         bias=shrv[0:1, :], scale=invW)
            nc.sync.dma_start(cc_in[h].rearrange("(o e) -> o e", o=1),
                              stg[h][:])
            nc.gpsimd.collective_compute(
                "AllReduce", mybir.AluOpType.add, replica_groups=rg,
                ins=[cc_in[h]], outs=[cc_out[h]])

        def half_update(k, h, src):
            """bcast s -> t = x0*s -> X' = g*src + t (+stats / final store)."""
            last = k == NL - 1
            nc.sync.dma_start(
                s_sb[h][:],
                cc_out[h].rearrange("(o e) -> o e", o=1).partition_broadcast(P))
            for cc in range(HCH):
                c = h * HCH + cc
                s_ap = s_sb[h][:, cc * CH:(cc + 1) * CH]
                tt = tp.tile([P, CH], fp16, tag="tt", name="tt")
                nc.vector.tensor_tensor(tt[:], X0[c][:], s_ap, alu.mult)
                nc.vector.tensor_scalar(X[c][:], src[c][:], geff[:], 0.0,
                                        alu.mult, alu.add)
                eng = nc.gpsimd if cc % 3 == 2 else nc.vector
                eng.tensor_tensor(X[c][:], X[c][:], tt[:], alu.add)
                if last:
                    st = io.tile([P, CH], f32, tag="io", name="io")
                    nc.scalar.activation(st[:], X[c][:], AF.Identity,
                                         bias=Dof[:], scale=1.0)
                    nc.sync.dma_start(outT[:, c * CH:(c + 1) * CH], st[:])
                else:
                    nc.vector.tensor_scalar(edump[:], X[c][:], 1.0, 0.0,
                                            alu.mult, alu.add,
                                            accum_out=ts_acc[:, c:c + 1])
                    nc.scalar.activation(fdump[:], X[c][:], AF.Square,
                                         accum_out=sq_acc[:, c:c + 1])

        for k in range(NL):
            src = X0 if k == 0 else X
            layer_params(k)
            half_dots(k, 0, src)
            half_dots(k, 1, src)     # AR(h1) flies while update(h0) runs
            half_update(k, 0, src)
            half_update(k, 1, src)
            if k < NL - 1:
                finalize_stats()

    return nc


_CACHE = {}


def _get_compiled():
    if "nc" not in _CACHE:
        nc = build_nc()
        nc.compile()
        _CACHE["nc"] = nc
    return _CACHE["nc"]


def kernel(x, gamma, beta, w, b):
    from concourse.bass_utils import run_bass_kernel_spmd

    x = np.asarray(x, dtype=np.float32)
    gamma = np.asarray(gamma, dtype=np.float32)
    beta = np.asarray(beta, dtype=np.float32)
    w = np.asarray(w, dtype=np.float32)
    b = np.asarray(b, dtype=np.float32)
    B_, L_ = x.shape

    nc = _get_compiled()
    in_maps = []
    for c in range(NCORES):
        cols = slice(c * P, (c + 1) * P)
        in_maps.append({
            "xT": np.ascontiguousarray(x[:, cols].T),
            "par": np.ascontiguousarray(np.concatenate(
                [gamma[:, cols].T, beta[:, cols].T,
                 w[:, cols].T, b[:, cols].T], axis=1)),
        })
    res = run_bass_kernel_spmd(nc, in_maps, list(range(NCORES))).results
    out = np.empty((B_, L_), np.float32)
    for c in range(NCORES):
        out[:, c * P:(c + 1) * P] = res[c]["outT"].T
    return out
